# revision 37
# baseline (speedup 1.0000x reference)
"""Bahdanau-style attention kernel for Trainium2 (8 NeuronCores, data-parallel).

Computes, for each batch b:
    h_proj = hidden @ w_h^T + attn_b                  # [H]
    e_proj = enc[b] @ w_e^T                           # [L, H]
    energy = tanh(h_proj + e_proj)                    # [L, H]
    scores = energy @ v_w                             # [L]
    weights = softmax(scores)                         # [L]
    context[b] = weights @ enc[b]                     # [B, H]

Sharding: data-parallel over batch B=32 across 8 cores (4 batches/core).
Params replicated. Softmax max-subtraction skipped (scores bounded by
sum|v| <= 32); the 1/Z normalization is folded into the final scaling.

The dominant e_proj GEMM runs in fp8 e4m3 with MatmulPerfMode.DoubleRow:
each matmul contracts TWO 128-deep k-tiles per pass (lhsT [128,2,128],
rhs [128,2,N]), streaming fp8 at the PE's full moving-port bandwidth -
2x the fp32r/bf16 rate. enc and w_e are quantized to e4m3 on the host
(input rel-err tolerance is 2e-2; fp8 contributes ~1%). The context
GEMM keeps bf16 enc (fp8 weights there would breach tolerance).

Processing is organized in double-slabs of 1024 l-positions so the tanh
runs on [128, 1024] two-bank PSUM tiles (halves per-instruction Act
overhead) while keeping the per-(o,b) h_proj bias constant per
activation. The v-weighted partition accumulation runs on DVE in bf16
(2x DVE rate); scores are partition-reduced by a ones-matmul, exp'd on
Act (bf16 out, fp32 Z accumulation), transposed via a DRAM bounce, and
the context matmuls run in bf16 with fp32 PSUM accumulation.

Schedule notes (from HW traces): each step's trailing ctx matmuls are
issued BEFORE the score ones-matmul so the PE stays fed through the
serial act->stt tail; dummy Tanh/Exp activations at init pre-load the
activation table; the w_e load goes out as one contiguous 8KB/line DMA
(chunked 2KB-line loads ran at ~60GB/s); the final slab splits its
exp/bounce/ctx per 512-half to shorten the pipeline drain. All DMA
queues share one engine, so startup is bound by the ~2MB of w_e+encT
the first matmuls need (~24us to first DR matmul incl. ~9us fixed
preamble). Steady state runs the PE matmul pipe at ~96% occupancy.

Built on bacc.Bacc so compile() runs the TRN2 wait-splitting passes
(move_matmul_waits_to_ldweights / generate_event_semaphores).
"""

import numpy as np

H = 1024
B = 32
L = 2048
NCORES = 8
BPC = B // NCORES          # batches per core = 4
OC = H // 128              # output-feature chunks = 8
KP = H // 256              # contraction k-PAIRS (DoubleRow) = 4
NDS = L // 1024            # double-slabs of 1024 l per batch = 2
NLT = L // 512             # encN tiles of 512 l = 4
NLCH = L // 128            # l-chunks of 128 = 16

_CACHED_NC = None


def _build_kernel():
    from contextlib import ExitStack

    import concourse.tile as tile
    from concourse import bacc
    from concourse import mybir
    from concourse.masks import make_identity

    f32 = mybir.dt.float32
    f32r = mybir.dt.float32r
    f8 = mybir.dt.float8e4
    bf16 = mybir.dt.bfloat16
    AF = mybir.ActivationFunctionType
    DR = mybir.MatmulPerfMode.DoubleRow

    nc = bacc.Bacc("TRN2", target_bir_lowering=False, debug=False,
                   num_devices=NCORES)

    # all inputs host-laid-out so every DMA is contiguous per partition
    encT = nc.dram_tensor("encTr", [BPC, 128, NDS, KP, 2, 1024], f8,
                          kind="ExternalInput").ap()
    encN = nc.dram_tensor("encNr", [BPC, 128, NLT, 4, H], bf16,
                          kind="ExternalInput").ap()
    w_eT = nc.dram_tensor("wer", [128, KP, 2, H], f8,
                          kind="ExternalInput").ap()
    smallr = nc.dram_tensor("smallr", [128, OC + OC * BPC], f32,
                            kind="ExternalInput").ap()
    ctx_out = nc.dram_tensor("ctx", [BPC, H], f32, kind="ExternalOutput").ap()
    # DRAM bounce buffer used to transpose exp(scores) [1,1024] -> [128,8]
    escr = nc.dram_tensor("escr", [BPC, L], bf16).ap()

    with tile.TileContext(nc) as tc, ExitStack() as ctx:
        consts = ctx.enter_context(tc.tile_pool(name="consts", bufs=1))
        encT_pool = ctx.enter_context(tc.tile_pool(name="encT", bufs=3))
        encN_pool = ctx.enter_context(tc.tile_pool(name="encN", bufs=4))
        en_pool = ctx.enter_context(tc.tile_pool(name="energy", bufs=4))
        acc_pool = ctx.enter_context(tc.tile_pool(name="acc", bufs=2))
        dacc_pool = ctx.enter_context(tc.tile_pool(name="dacc", bufs=2))
        small = ctx.enter_context(tc.tile_pool(name="small", bufs=2))
        expwT_pool = ctx.enter_context(tc.tile_pool(name="expwT", bufs=2))

        # ---- constants ----
        # tiny consts + first enc slab on sync; the w_e load rides the
        # scalar queue in parallel.
        # startup is DMA-latency-bound: sync streams the first enc slab while
        # scalar brings the small consts + the whole w_e in one contiguous
        # transfer (8KB/partition lines - small chunked loads ran at 60GB/s)
        encTs_pre = encT_pool.tile([128, KP, 2, 1024], f8, tag="encTs",
                                   name="encTs_pre")
        nc.sync.dma_start(out=encTs_pre, in_=encT[0, :, 0])
        small_sb = consts.tile([128, OC + OC * BPC], f32)
        nc.scalar.dma_start(out=small_sb, in_=smallr)
        v_sb = small_sb[:, 0:OC]
        # h_proj + attn_b, host-folded: [128, OC, BPC]
        hproj_sb = small_sb[:, OC:].rearrange("p (o b) -> p o b", b=BPC)
        we_sb = consts.tile([128, KP, 2, H], f8)     # w_e^T  [h-part, kp, i, o]
        nc.scalar.dma_start(out=we_sb, in_=w_eT)
        ident = consts.tile([128, 128], f32)
        make_identity(nc, ident)
        ones_f32 = consts.tile([128, 1], f32)
        nc.vector.memset(ones_f32, 1.0)
        ones_bf = consts.tile([128, 1], bf16)
        nc.vector.tensor_copy(ones_bf, ones_f32)
        ones_fr = consts.tile([128, 1], mybir.dt.float32r)
        nc.vector.tensor_copy(ones_fr, ones_f32)
        v_bf = consts.tile([128, OC], bf16)
        nc.vector.tensor_copy(v_bf, v_sb)
        # dummy activations so the Tanh/Exp table loads overlap the warmup
        # instead of stalling the first real tanh (~2us table load)
        dummy = consts.tile([1, 1], f32)
        nc.scalar.activation(dummy, ones_f32[0:1, :], AF.Tanh)
        nc.scalar.activation(dummy, ones_f32[0:1, :], AF.Exp)

        with tc.tile_pool(name="pp_pro", bufs=1, space="PSUM") as pp_pro:
            # warm the PE HAM while the weight DMAs stream
            # enough ident matmuls to keep the PE clock at full p-state
            # through the DMA-bound startup window (~24us to first DR mm);
            # after a ~10us idle gap the first real matmuls ran at half rate
            pwarm = pp_pro.tile([128, 128], f32, tag="pwarm")
            for w in range(60):
                nc.tensor.matmul(pwarm, ident, ident, start=True, stop=True,
                                 skip_group_check=True)

        pp_e = ctx.enter_context(tc.tile_pool(name="pp_e", bufs=2, space="PSUM"))
        pp_s = ctx.enter_context(tc.tile_pool(name="pp_s", bufs=1, space="PSUM"))
        pp_c = ctx.enter_context(tc.tile_pool(name="pp_c", bufs=2, space="PSUM"))

        # ---- main pipeline: flat stream of double-slabs across batches ----
        # ctx matmuls run one slab behind their exp-bounce; batch finalization
        # (Z reduce + scale + store) rides behind the next batch's first slab.
        state = {}

        # batches 0..BPC-2 offload 3 of 8 ctx chunks per double-slab to DVE
        # (fp32 accumulate; folded into the pcs psum group at finalize). The
        # last batch stays all-PE so the pipeline drain isn't lengthened.
        DVE_JS = ()

        def ctx_mms(b, ds, encNs2, js=range(OC)):
            st = state[b]
            has_dve = bool(DVE_JS) and b < BPC - 1
            if st["pcs"] is None:
                st["pcs"] = [pp_c.tile([1, 512], f32, tag="pc",
                                       name=f"pc{b}_{i}") for i in range(2)]
            if has_dve and DVE_JS and DVE_JS[0] in js:
                # DVE stt scalars must be fp32; upconvert the needed columns
                lo = ds * OC + DVE_JS[0]
                wc32 = small.tile([128, len(DVE_JS)], f32, tag="wc32",
                                  name=f"wc32_{b}_{ds}")
                nc.vector.tensor_copy(
                    wc32, st["expwT"][:, lo:lo + len(DVE_JS)])
            for half, j in [(h, jj) for h in range(2) for jj in js]:
                lc = ds * OC + j
                encNs = encNs2[j // 4]
                if has_dve and j in DVE_JS:
                    w_col = wc32[:, j - DVE_JS[0]:j - DVE_JS[0] + 1]
                    if st["dacc"] is None:
                        st["dacc"] = dacc_pool.tile([128, H], f32r,
                                                    tag="dacc",
                                                    name=f"dacc{b}")
                        nc.vector.tensor_scalar_mul(
                            st["dacc"], encNs[:, j % 4, :], w_col)
                    else:
                        nc.vector.scalar_tensor_tensor(
                            out=st["dacc"], in0=encNs[:, j % 4, :],
                            scalar=w_col, in1=st["dacc"],
                            op0=mybir.AluOpType.mult,
                            op1=mybir.AluOpType.add)
                    continue
                nc.tensor.matmul(
                    st["pcs"][half],
                    st["expwT"][:, lc:lc + 1],
                    encNs[:, j % 4, half * 512:(half + 1) * 512],
                    start=(lc == 0),
                    stop=(lc == NLCH - 1 and not has_dve),
                )

        def finalize(b):
            st = state.pop(b)
            if st["dacc"] is not None:
                # fold the DVE-accumulated ctx chunks into the psum group
                # (f32r moving at 1 cyc/col); these close the group
                for half in range(2):
                    nc.tensor.matmul(
                        st["pcs"][half], ones_fr,
                        st["dacc"][:, half * 512:(half + 1) * 512],
                        start=False, stop=True)
            zs = small.tile([1, 1], f32, tag="zs", name=f"zs{b}")
            nc.vector.reduce_sum(zs, st["zacc"], axis=mybir.AxisListType.X)
            rz = small.tile([1, 1], f32, tag="rz", name=f"rz{b}")
            nc.vector.reciprocal(rz, zs)
            ctx_sb = small.tile([1, H], f32, tag="ctx", name=f"ctx{b}")
            for half in range(2):
                nc.vector.tensor_scalar_mul(
                    ctx_sb[:, half * 512:(half + 1) * 512],
                    st["pcs"][half], rz)
            eng = nc.sync if b == BPC - 1 else nc.gpsimd
            eng.dma_start(out=ctx_out[b:b + 1, :], in_=ctx_sb)

        pending = []
        fin_pending = []
        for s in range(BPC * NDS):
            b, ds = divmod(s, NDS)
            if ds == 0:
                # the last batch's final slab splits its exp into halves
                # (one extra zacc slot) to shorten the pipeline drain
                nz = NDS + 1 if b == BPC - 1 else NDS
                state[b] = {
                    "expwT": expwT_pool.tile([128, NLCH], bf16, tag="expwT",
                                             name=f"expwT{b}"),
                    "pcs": None,
                    "dacc": None,
                    "zacc": small.tile([1, nz], f32, tag="zacc",
                                       name=f"zacc{b}"),
                }
            st = state[b]
            last = s == BPC * NDS - 1

            if s == 0:
                encTs = encTs_pre
            else:
                encTs = encT_pool.tile([128, KP, 2, 1024], f8, tag="encTs")
                nc.sync.dma_start(out=encTs, in_=encT[b, :, ds])
            # encN loads ride the (nearly idle) gpsimd queue, issued at step
            # top so they're resident well before the trailing ctx matmuls.
            encNs2 = []
            for i in range(2):
                encNs = encN_pool.tile([128, 4, H], bf16, tag="encNs",
                                       name=f"encNs{b}_{ds}_{i}")
                nc.gpsimd.dma_start(out=encNs, in_=encN[b, :, ds * 2 + i])
                encNs2.append(encNs)
            # energy accumulation in bf16 on DVE (2x rate); acc[p, l] =
            # sum_o v[p, o] * tanh(...). The LAST step instead reduces via
            # PE v-matmuls trailing one activation behind, cutting the
            # serial DVE stt + ones-mm legs out of the pipeline drain.
            acc = None if last else acc_pool.tile([128, 1024], bf16,
                                                  tag="acc")
            psum_sc = pp_s.tile([1, 1024], f32, tag="psc")
            ens = []

            def vmms(o):
                for half in range(2):
                    nc.tensor.matmul(
                        psum_sc[:, half * 512:(half + 1) * 512],
                        v_bf[:, o:o + 1],
                        ens[o][:, half * 512:(half + 1) * 512],
                        start=(o == 0), stop=(o == OC - 1),
                    )

            for o in range(OC):
                pe = pp_e.tile([128, 1024], f32, tag="pe")
                for kp in range(KP):
                    for half in range(2):
                        nc.tensor.matmul(
                            pe[:, half * 512:(half + 1) * 512],
                            we_sb[:, kp, :, o * 128:(o + 1) * 128],
                            encTs[:, kp, :, half * 512:(half + 1) * 512],
                            start=(kp == 0), stop=(kp == KP - 1),
                            perf_mode=DR,
                        )
                if last and o >= 1:
                    vmms(o - 1)
                en = en_pool.tile([128, 1024], bf16, tag="en")
                nc.scalar.activation(en, pe, AF.Tanh,
                                     bias=hproj_sb[:, o, b:b + 1])
                ens.append(en)
                if last:
                    continue
                if o == 0:
                    nc.vector.tensor_scalar_mul(acc, en, v_sb[:, 0:1])
                else:
                    nc.vector.scalar_tensor_tensor(
                        out=acc, in0=en, scalar=v_sb[:, o:o + 1], in1=acc,
                        op0=mybir.AluOpType.mult, op1=mybir.AluOpType.add)
            # trailing ctx matmuls go here, BEFORE the ones-matmul: the
            # ones-mm waits on the serial act->stt tail (~2.5us past the last
            # big mm group), and the ctx mms keep the PE fed through it.
            pending.append((b, ds, encNs2))
            if len(pending) > 1:
                pb, pds, pencNs2 = pending.pop(0)
                ctx_mms(pb, pds, pencNs2)
                if pds == NDS - 1:
                    finalize(pb)
            # partition reduction of acc via ones-matmul (fp32 psum), then
            # exp (no max subtraction; scores bounded) with Z-part for free,
            # then transpose exp(scores) into [l-part, chunk] layout via a
            # DRAM bounce. The final slab runs this per 512-half on the idle
            # sync queue and issues its own ctx matmuls inline, so the drain
            # chain overlaps itself; earlier slabs do the full 1024 on
            # gpsimd so the enc stream is never blocked.
            if last:
                vmms(OC - 1)
            for half in range(2):
                if not last:
                    nc.tensor.matmul(
                        psum_sc[:, half * 512:(half + 1) * 512], ones_bf,
                        acc[:, half * 512:(half + 1) * 512],
                        start=True, stop=True)
                    continue
                expw = small.tile([1, 512], bf16, tag="expwh",
                                  name=f"expw{s}_{half}")
                nc.scalar.activation(expw,
                                     psum_sc[:, half * 512:(half + 1) * 512],
                                     AF.Exp,
                                     accum_out=st["zacc"][:, ds + half:
                                                          ds + half + 1])
                off = ds * 1024 + half * 512
                wr_eng = rd_eng = nc.sync
                wr_eng.dma_start(out=escr[b:b + 1, off:off + 512], in_=expw)
                rd_eng.dma_start(
                    out=st["expwT"][:, ds * OC + half * 4:
                                    ds * OC + (half + 1) * 4],
                    in_=escr[b, off:off + 512]
                    .rearrange("(c p) -> p c", p=128),
                )
                ctx_mms(b, ds, encNs2,
                        js=range(half * 4, (half + 1) * 4))
            if last:
                finalize(b)
                continue
            expw = small.tile([1, 1024], bf16, tag="expw")
            nc.scalar.activation(expw, psum_sc, AF.Exp,
                                 accum_out=st["zacc"][:, ds:ds + 1])
            nc.gpsimd.dma_start(
                out=escr[b:b + 1, ds * 1024:(ds + 1) * 1024], in_=expw)
            nc.gpsimd.dma_start(
                out=st["expwT"][:, ds * OC:(ds + 1) * OC],
                in_=escr[b, ds * 1024:(ds + 1) * 1024]
                .rearrange("(c p) -> p c", p=128),
            )

    nc.compile()
    return nc


def _get_nc():
    global _CACHED_NC
    if _CACHED_NC is None:
        _CACHED_NC = _build_kernel()
    return _CACHED_NC


def _make_in_maps(hidden, encoder_outputs, attn_w, attn_b, v_w):
    import ml_dtypes

    f8 = ml_dtypes.float8_e4m3
    bf16 = ml_dtypes.bfloat16

    hidden = np.asarray(hidden, dtype=np.float32)
    encoder_outputs = np.asarray(encoder_outputs, dtype=np.float32)
    attn_w = np.asarray(attn_w, dtype=np.float32)
    attn_b = np.asarray(attn_b, dtype=np.float32)
    v_w = np.asarray(v_w, dtype=np.float32)

    # w_e^T in fp8, DoubleRow pair layout: wer[p, kp, i, o] =
    # w_e[o, kp*256 + i*128 + p]
    wer = np.ascontiguousarray(
        attn_w[:, H:].T.reshape(KP, 2, 128, H).transpose(2, 0, 1, 3)
    ).astype(f8)
    # fold the tiny h_proj = hidden @ w_h^T + b into a per-core bias input
    hproj_pb = hidden @ attn_w[:, :H].T + attn_b     # [B, H]

    in_maps = []
    for c in range(NCORES):
        sl = slice(c * BPC, (c + 1) * BPC)
        enc = encoder_outputs[sl]                       # [BPC, L, H]
        enc8 = enc.astype(f8)
        # encTr[b, p, ds, kp, i, l] = enc[b, ds*1024 + l, kp*256 + i*128 + p]
        encTr = np.ascontiguousarray(
            enc8.reshape(BPC, NDS, 1024, KP, 2, 128)
            .transpose(0, 5, 1, 3, 4, 2))
        # encNr[b, p, lt, j, h] = enc[b, lt*512 + j*128 + p, h]
        encNr = np.ascontiguousarray(
            enc.reshape(BPC, NLT, 4, 128, H).transpose(0, 3, 1, 2, 4)
            .astype(bf16))
        # smallr: [v chunks | h_proj+b chunks]  (hp[p, o, b] layout)
        hp = hproj_pb[sl].T.reshape(OC, 128, BPC).transpose(1, 0, 2)
        smallr = np.concatenate([
            v_w.reshape(OC, 128).T,
            hp.reshape(128, OC * BPC),
        ], axis=1)
        in_maps.append({
            "encTr": encTr,
            "encNr": encNr,
            "wer": wer,
            "smallr": np.ascontiguousarray(smallr),
        })
    return in_maps


def kernel(hidden, encoder_outputs, attn_w, attn_b, v_w):
    from concourse.bass_utils import run_bass_kernel_spmd

    in_maps = _make_in_maps(hidden, encoder_outputs, attn_w, attn_b, v_w)
    nc = _get_nc()
    res = run_bass_kernel_spmd(nc, in_maps, list(range(NCORES)))
    out = np.concatenate([res.results[c]["ctx"] for c in range(NCORES)], axis=0)
    return out.astype(np.float32)


# revision 38
# speedup vs baseline: 1.0281x; 1.0281x over previous
"""Bahdanau-style attention kernel for Trainium2 (8 NeuronCores, data-parallel).

Computes, for each batch b:
    h_proj = hidden @ w_h^T + attn_b                  # [H]
    e_proj = enc[b] @ w_e^T                           # [L, H]
    energy = tanh(h_proj + e_proj)                    # [L, H]
    scores = energy @ v_w                             # [L]
    weights = softmax(scores)                         # [L]
    context[b] = weights @ enc[b]                     # [B, H]

Sharding: data-parallel over batch B=32 across 8 cores (4 batches/core).
Params replicated. Softmax max-subtraction skipped (scores bounded by
sum|v| <= 32); the 1/Z normalization is folded into the final scaling.

The dominant e_proj GEMM runs in fp8 e4m3 with MatmulPerfMode.DoubleRow:
each matmul contracts TWO 128-deep k-tiles per pass (lhsT [128,2,128],
rhs [128,2,N]), streaming fp8 at the PE's full moving-port bandwidth -
2x the fp32r/bf16 rate. enc and w_e are quantized to e4m3 on the host
(input rel-err tolerance is 2e-2; fp8 contributes ~1%). The context
GEMM keeps bf16 enc (fp8 weights there would breach tolerance).

Processing is organized in double-slabs of 1024 l-positions so the tanh
runs on [128, 1024] two-bank PSUM tiles (halves per-instruction Act
overhead) while keeping the per-(o,b) h_proj bias constant per
activation. The v-weighted partition accumulation runs on DVE in bf16
(2x DVE rate); scores are partition-reduced by a ones-matmul, exp'd on
Act (bf16 out, fp32 Z accumulation), transposed via a DRAM bounce, and
the context matmuls run in bf16 with fp32 PSUM accumulation.

Schedule notes (from HW traces): each step's trailing ctx matmuls are
issued BEFORE the score ones-matmul so the PE stays fed through the
serial act->stt tail; dummy Tanh/Exp activations at init pre-load the
activation table; the w_e load goes out as one contiguous 8KB/line DMA
(chunked 2KB-line loads ran at ~60GB/s); the final slab splits its
exp/bounce/ctx per 512-half to shorten the pipeline drain. All DMA
queues share one engine, so startup is bound by the ~2MB of w_e+encT
the first matmuls need (~24us to first DR matmul incl. ~9us fixed
preamble). Steady state runs the PE matmul pipe at ~96% occupancy.

Built on bacc.Bacc so compile() runs the TRN2 wait-splitting passes
(move_matmul_waits_to_ldweights / generate_event_semaphores).
"""

import numpy as np

H = 1024
B = 32
L = 2048
NCORES = 8
BPC = B // NCORES          # batches per core = 4
OC = H // 128              # output-feature chunks = 8
KP = H // 256              # contraction k-PAIRS (DoubleRow) = 4
NDS = L // 1024            # double-slabs of 1024 l per batch = 2
NLT = L // 512             # encN tiles of 512 l = 4
NLCH = L // 128            # l-chunks of 128 = 16

_CACHED_NC = None


def _build_kernel():
    from contextlib import ExitStack

    import concourse.tile as tile
    from concourse import bacc
    from concourse import mybir
    from concourse.masks import make_identity

    f32 = mybir.dt.float32
    f32r = mybir.dt.float32r
    f8 = mybir.dt.float8e4
    bf16 = mybir.dt.bfloat16
    AF = mybir.ActivationFunctionType
    DR = mybir.MatmulPerfMode.DoubleRow

    nc = bacc.Bacc("TRN2", target_bir_lowering=False, debug=False,
                   num_devices=NCORES)

    # all inputs host-laid-out so every DMA is contiguous per partition
    encT = nc.dram_tensor("encTr", [BPC, 128, NDS, KP, 2, 1024], f8,
                          kind="ExternalInput").ap()
    encN = nc.dram_tensor("encNr", [BPC, 128, NLT, 4, H], bf16,
                          kind="ExternalInput").ap()
    w_eT = nc.dram_tensor("wer", [128, KP, 2, H], f8,
                          kind="ExternalInput").ap()
    smallr = nc.dram_tensor("smallr", [128, OC + OC * BPC], f32,
                            kind="ExternalInput").ap()
    ctx_out = nc.dram_tensor("ctx", [BPC, H], f32, kind="ExternalOutput").ap()
    # DRAM bounce buffer used to transpose exp(scores) [1,1024] -> [128,8]
    escr = nc.dram_tensor("escr", [BPC, L], bf16).ap()

    with tile.TileContext(nc) as tc, ExitStack() as ctx:
        consts = ctx.enter_context(tc.tile_pool(name="consts", bufs=1))
        encT_pool = ctx.enter_context(tc.tile_pool(name="encT", bufs=3))
        encN_pool = ctx.enter_context(tc.tile_pool(name="encN", bufs=4))
        en_pool = ctx.enter_context(tc.tile_pool(name="energy", bufs=4))
        acc_pool = ctx.enter_context(tc.tile_pool(name="acc", bufs=2))
        dacc_pool = ctx.enter_context(tc.tile_pool(name="dacc", bufs=2))
        small = ctx.enter_context(tc.tile_pool(name="small", bufs=2))
        expwT_pool = ctx.enter_context(tc.tile_pool(name="expwT", bufs=2))

        # ---- constants ----
        # tiny consts + first enc slab on sync; the w_e load rides the
        # scalar queue in parallel.
        # startup is DMA-latency-bound: sync streams the first enc slab while
        # scalar brings the small consts + the whole w_e in one contiguous
        # transfer (8KB/partition lines - small chunked loads ran at 60GB/s)
        encTs_pre = encT_pool.tile([128, KP, 2, 1024], f8, tag="encTs",
                                   name="encTs_pre")
        nc.sync.dma_start(out=encTs_pre, in_=encT[0, :, 0])
        small_sb = consts.tile([128, OC + OC * BPC], f32)
        nc.scalar.dma_start(out=small_sb, in_=smallr)
        v_sb = small_sb[:, 0:OC]
        # h_proj + attn_b, host-folded: [128, OC, BPC]
        hproj_sb = small_sb[:, OC:].rearrange("p (o b) -> p o b", b=BPC)
        we_sb = consts.tile([128, KP, 2, H], f8)     # w_e^T  [h-part, kp, i, o]
        nc.scalar.dma_start(out=we_sb, in_=w_eT)
        ident = consts.tile([128, 128], f32)
        make_identity(nc, ident)
        ones_f32 = consts.tile([128, 1], f32)
        nc.vector.memset(ones_f32, 1.0)
        ones_bf = consts.tile([128, 1], bf16)
        nc.vector.tensor_copy(ones_bf, ones_f32)
        ones_fr = consts.tile([128, 1], mybir.dt.float32r)
        nc.vector.tensor_copy(ones_fr, ones_f32)
        v_bf = consts.tile([128, OC], bf16)
        nc.vector.tensor_copy(v_bf, v_sb)
        # dummy activations so the Tanh/Exp table loads overlap the warmup
        # instead of stalling the first real tanh (~2us table load)
        dummy = consts.tile([1, 1], f32)
        nc.scalar.activation(dummy, ones_f32[0:1, :], AF.Tanh)
        nc.scalar.activation(dummy, ones_f32[0:1, :], AF.Exp)

        with tc.tile_pool(name="pp_pro", bufs=1, space="PSUM") as pp_pro:
            # warm the PE HAM while the weight DMAs stream
            # enough ident matmuls to keep the PE clock at full p-state
            # through the DMA-bound startup window (~24us to first DR mm);
            # after a ~10us idle gap the first real matmuls ran at half rate
            pwarm = pp_pro.tile([128, 128], f32, tag="pwarm")
            for w in range(60):
                nc.tensor.matmul(pwarm, ident, ident, start=True, stop=True,
                                 skip_group_check=True)

        pp_e = ctx.enter_context(tc.tile_pool(name="pp_e", bufs=2, space="PSUM"))
        pp_s = ctx.enter_context(tc.tile_pool(name="pp_s", bufs=1, space="PSUM"))
        pp_c = ctx.enter_context(tc.tile_pool(name="pp_c", bufs=2, space="PSUM"))

        # ---- main pipeline: flat stream of double-slabs across batches ----
        # ctx matmuls run one slab behind their exp-bounce; batch finalization
        # (Z reduce + scale + store) rides behind the next batch's first slab.
        state = {}

        # batches 0..BPC-2 offload 3 of 8 ctx chunks per double-slab to DVE
        # (fp32 accumulate; folded into the pcs psum group at finalize). The
        # last batch stays all-PE so the pipeline drain isn't lengthened.
        DVE_JS = ()

        def ctx_mms(b, ds, encNs2, js=range(OC)):
            st = state[b]
            has_dve = bool(DVE_JS) and b < BPC - 1
            if st["pcs"] is None:
                st["pcs"] = [pp_c.tile([1, 512], f32, tag="pc",
                                       name=f"pc{b}_{i}") for i in range(2)]
            if has_dve and DVE_JS and DVE_JS[0] in js:
                # DVE stt scalars must be fp32; upconvert the needed columns
                lo = ds * OC + DVE_JS[0]
                wc32 = small.tile([128, len(DVE_JS)], f32, tag="wc32",
                                  name=f"wc32_{b}_{ds}")
                nc.vector.tensor_copy(
                    wc32, st["expwT"][:, lo:lo + len(DVE_JS)])
            for half, j in [(h, jj) for h in range(2) for jj in js]:
                lc = ds * OC + j
                encNs = encNs2[j // 4]
                if has_dve and j in DVE_JS:
                    w_col = wc32[:, j - DVE_JS[0]:j - DVE_JS[0] + 1]
                    if st["dacc"] is None:
                        st["dacc"] = dacc_pool.tile([128, H], f32r,
                                                    tag="dacc",
                                                    name=f"dacc{b}")
                        nc.vector.tensor_scalar_mul(
                            st["dacc"], encNs[:, j % 4, :], w_col)
                    else:
                        nc.vector.scalar_tensor_tensor(
                            out=st["dacc"], in0=encNs[:, j % 4, :],
                            scalar=w_col, in1=st["dacc"],
                            op0=mybir.AluOpType.mult,
                            op1=mybir.AluOpType.add)
                    continue
                nc.tensor.matmul(
                    st["pcs"][half],
                    st["expwT"][:, lc:lc + 1],
                    encNs[:, j % 4, half * 512:(half + 1) * 512],
                    start=(lc == 0),
                    stop=(lc == NLCH - 1 and not has_dve),
                )

        def finalize(b):
            st = state.pop(b)
            if st["dacc"] is not None:
                # fold the DVE-accumulated ctx chunks into the psum group
                # (f32r moving at 1 cyc/col); these close the group
                for half in range(2):
                    nc.tensor.matmul(
                        st["pcs"][half], ones_fr,
                        st["dacc"][:, half * 512:(half + 1) * 512],
                        start=False, stop=True)
            zs = small.tile([1, 1], f32, tag="zs", name=f"zs{b}")
            nc.vector.reduce_sum(zs, st["zacc"], axis=mybir.AxisListType.X)
            rz = small.tile([1, 1], f32, tag="rz", name=f"rz{b}")
            nc.vector.reciprocal(rz, zs)
            ctx_sb = small.tile([1, H], f32, tag="ctx", name=f"ctx{b}")
            for half in range(2):
                nc.vector.tensor_scalar_mul(
                    ctx_sb[:, half * 512:(half + 1) * 512],
                    st["pcs"][half], rz)
            eng = nc.sync if b == BPC - 1 else nc.gpsimd
            eng.dma_start(out=ctx_out[b:b + 1, :], in_=ctx_sb)

        pending = []
        fin_pending = []
        for s in range(BPC * NDS):
            b, ds = divmod(s, NDS)
            if ds == 0:
                # the last batch's final slab splits its exp into halves
                # (one extra zacc slot) to shorten the pipeline drain
                nz = NDS + 1 if b == BPC - 1 else NDS
                state[b] = {
                    "expwT": expwT_pool.tile([128, NLCH], bf16, tag="expwT",
                                             name=f"expwT{b}"),
                    "pcs": None,
                    "dacc": None,
                    "zacc": small.tile([1, nz], f32, tag="zacc",
                                       name=f"zacc{b}"),
                }
            st = state[b]
            last = s == BPC * NDS - 1

            if s == 0:
                encTs = encTs_pre
            else:
                encTs = encT_pool.tile([128, KP, 2, 1024], f8, tag="encTs")
                nc.sync.dma_start(out=encTs, in_=encT[b, :, ds])
            # encN loads ride the (nearly idle) gpsimd queue, issued at step
            # top so they're resident well before the trailing ctx matmuls.
            encNs2 = []
            for i in range(2):
                encNs = encN_pool.tile([128, 4, H], bf16, tag="encNs",
                                       name=f"encNs{b}_{ds}_{i}")
                nc.gpsimd.dma_start(out=encNs, in_=encN[b, :, ds * 2 + i])
                encNs2.append(encNs)
            # energy accumulation in bf16 on DVE (2x rate); acc[p, l] =
            # sum_o v[p, o] * tanh(...)
            acc = acc_pool.tile([128, 1024], bf16, tag="acc")
            for o in range(OC):
                pe = pp_e.tile([128, 1024], f32, tag="pe")
                for kp in range(KP):
                    for half in range(2):
                        nc.tensor.matmul(
                            pe[:, half * 512:(half + 1) * 512],
                            we_sb[:, kp, :, o * 128:(o + 1) * 128],
                            encTs[:, kp, :, half * 512:(half + 1) * 512],
                            start=(kp == 0), stop=(kp == KP - 1),
                            perf_mode=DR,
                        )
                en = en_pool.tile([128, 1024], bf16, tag="en")
                nc.scalar.activation(en, pe, AF.Tanh,
                                     bias=hproj_sb[:, o, b:b + 1])
                if o == 0:
                    nc.vector.tensor_scalar_mul(acc, en, v_sb[:, 0:1])
                else:
                    nc.vector.scalar_tensor_tensor(
                        out=acc, in0=en, scalar=v_sb[:, o:o + 1], in1=acc,
                        op0=mybir.AluOpType.mult, op1=mybir.AluOpType.add)
            # trailing ctx matmuls go here, BEFORE the ones-matmul: the
            # ones-mm waits on the serial act->stt tail (~2.5us past the last
            # big mm group), and the ctx mms keep the PE fed through it.
            pending.append((b, ds, encNs2))
            if len(pending) > 1:
                pb, pds, pencNs2 = pending.pop(0)
                ctx_mms(pb, pds, pencNs2)
                if pds == NDS - 1:
                    finalize(pb)
            # partition reduction of acc via ones-matmul (fp32 psum), then
            # exp (no max subtraction; scores bounded) with Z-part for free,
            # then transpose exp(scores) into [l-part, chunk] layout via a
            # DRAM bounce. The final slab runs this per 512-half on the idle
            # sync queue and issues its own ctx matmuls inline, so the drain
            # chain overlaps itself; earlier slabs do the full 1024 on
            # gpsimd so the enc stream is never blocked.
            psum_sc = pp_s.tile([1, 1024], f32, tag="psc")
            for half in range(2):
                nc.tensor.matmul(psum_sc[:, half * 512:(half + 1) * 512],
                                 ones_bf,
                                 acc[:, half * 512:(half + 1) * 512],
                                 start=True, stop=True)
                if not last:
                    continue
                expw = small.tile([1, 512], bf16, tag="expwh",
                                  name=f"expw{s}_{half}")
                nc.scalar.activation(expw,
                                     psum_sc[:, half * 512:(half + 1) * 512],
                                     AF.Exp,
                                     accum_out=st["zacc"][:, ds + half:
                                                          ds + half + 1])
                off = ds * 1024 + half * 512
                wr_eng = rd_eng = nc.sync
                wr_eng.dma_start(out=escr[b:b + 1, off:off + 512], in_=expw)
                rd_eng.dma_start(
                    out=st["expwT"][:, ds * OC + half * 4:
                                    ds * OC + (half + 1) * 4],
                    in_=escr[b, off:off + 512]
                    .rearrange("(c p) -> p c", p=128),
                )
                ctx_mms(b, ds, encNs2,
                        js=range(half * 4, (half + 1) * 4))
            if last:
                finalize(b)
                continue
            expw = small.tile([1, 1024], bf16, tag="expw")
            nc.scalar.activation(expw, psum_sc, AF.Exp,
                                 accum_out=st["zacc"][:, ds:ds + 1])
            nc.gpsimd.dma_start(
                out=escr[b:b + 1, ds * 1024:(ds + 1) * 1024], in_=expw)
            nc.gpsimd.dma_start(
                out=st["expwT"][:, ds * OC:(ds + 1) * OC],
                in_=escr[b, ds * 1024:(ds + 1) * 1024]
                .rearrange("(c p) -> p c", p=128),
            )

    nc.compile()
    return nc


def _get_nc():
    global _CACHED_NC
    if _CACHED_NC is None:
        _CACHED_NC = _build_kernel()
    return _CACHED_NC


def _make_in_maps(hidden, encoder_outputs, attn_w, attn_b, v_w):
    import ml_dtypes

    f8 = ml_dtypes.float8_e4m3
    bf16 = ml_dtypes.bfloat16

    hidden = np.asarray(hidden, dtype=np.float32)
    encoder_outputs = np.asarray(encoder_outputs, dtype=np.float32)
    attn_w = np.asarray(attn_w, dtype=np.float32)
    attn_b = np.asarray(attn_b, dtype=np.float32)
    v_w = np.asarray(v_w, dtype=np.float32)

    # w_e^T in fp8, DoubleRow pair layout: wer[p, kp, i, o] =
    # w_e[o, kp*256 + i*128 + p]
    wer = np.ascontiguousarray(
        attn_w[:, H:].T.reshape(KP, 2, 128, H).transpose(2, 0, 1, 3)
    ).astype(f8)
    # fold the tiny h_proj = hidden @ w_h^T + b into a per-core bias input
    hproj_pb = hidden @ attn_w[:, :H].T + attn_b     # [B, H]

    in_maps = []
    for c in range(NCORES):
        sl = slice(c * BPC, (c + 1) * BPC)
        enc = encoder_outputs[sl]                       # [BPC, L, H]
        enc8 = enc.astype(f8)
        # encTr[b, p, ds, kp, i, l] = enc[b, ds*1024 + l, kp*256 + i*128 + p]
        encTr = np.ascontiguousarray(
            enc8.reshape(BPC, NDS, 1024, KP, 2, 128)
            .transpose(0, 5, 1, 3, 4, 2))
        # encNr[b, p, lt, j, h] = enc[b, lt*512 + j*128 + p, h]
        encNr = np.ascontiguousarray(
            enc.reshape(BPC, NLT, 4, 128, H).transpose(0, 3, 1, 2, 4)
            .astype(bf16))
        # smallr: [v chunks | h_proj+b chunks]  (hp[p, o, b] layout)
        hp = hproj_pb[sl].T.reshape(OC, 128, BPC).transpose(1, 0, 2)
        smallr = np.concatenate([
            v_w.reshape(OC, 128).T,
            hp.reshape(128, OC * BPC),
        ], axis=1)
        in_maps.append({
            "encTr": encTr,
            "encNr": encNr,
            "wer": wer,
            "smallr": np.ascontiguousarray(smallr),
        })
    return in_maps


def kernel(hidden, encoder_outputs, attn_w, attn_b, v_w):
    from concourse.bass_utils import run_bass_kernel_spmd

    in_maps = _make_in_maps(hidden, encoder_outputs, attn_w, attn_b, v_w)
    nc = _get_nc()
    res = run_bass_kernel_spmd(nc, in_maps, list(range(NCORES)))
    out = np.concatenate([res.results[c]["ctx"] for c in range(NCORES)], axis=0)
    return out.astype(np.float32)
